# revision 10
# baseline (speedup 1.0000x reference)
"""Trainium2 Bass kernel for batched weighted scatter-add (AttentionCopy).

Computes out[b, o, v] = sum_i attn[b, o, i] * (ids[b, i] == v)
for ids [16, 512] int32 in [0, 50000), attn [16, 32, 512] f32,
out [16, 32, 50000] f32.

Strategy: pure data parallel over the batch dim — 2 batches per core on 8
cores. Per batch the [32, 50000] output is built densely in 10 PSUM tiles of
[128, 1250], one per contiguous vocab span of 5000 = 4 groups x 1250. Tile
rows are o-major (o, gl) pairs (o in 0..31, gl in 0..3 local group), so each
tile's DRAM write is a [32, 4, 1250] access pattern whose outer dim (32)
spreads across all 16 SDMA engines (outer-dim count < 16 would leave engines
idle — measured 4x DMA slowdown with a g-major [4, 32, 1250] pattern).

The host buckets each batch's 512 ids into the 10 spans (index-only
preprocessing; uniform ids put ~51 of 512 in each span, max 67 observed,
capacity 128), gathers the matching attn columns, and ships the div/mod-1250
split (hi/lo) of the span-relative ids, so the device does a single K=128
matmul pass per tile instead of K=512 over all ids:

  out[(o, gl), lo] = gt.T @ alo,   gt[i, (o, gl)] = (hi_i == gl) * attnT[i, o]
                                   alo[i, lo]     = (lo_i == lo)

(hi = -1 marks padding slots; their gt columns are all zero.) This cuts
tensor-engine time ~4x (it was the bottleneck at 53us busy of 65us total),
leaving the kernel bounded by the mandatory 12.8 MB/core f32 output write
(~33us at the measured ~400 GB/s aggregate of the 16 SDMA engines).

The iota compare constants are generated on-device by gpsimd at t=0 (input
DMA is on the startup critical path; these were 352 KB of it). One-hot
builds run LA tiles ahead of the matmuls on the vector engine, PSUM->SBUF
copies rotate scalar/vector/gpsimd, and output DMAs are kicked on the two
HWDGE queues (scalar for scalar-copied tiles, sync for the rest).
"""

import sys

sys.path.insert(0, "/opt/trn_rl_repo")

import numpy as np

NCORES = 8
B, O, I = 16, 32, 512
SIZE = 50000
BPC = B // NCORES  # batches per core
V2 = 1250  # lo range (one output tile is 2.5 PSUM banks)
GPT = 4  # groups per output tile: 128 rows = 32 o x 4 groups
SPAN = GPT * V2  # 5000: vocab span per output tile
TILES = SIZE // SPAN  # 10 output tiles per batch
KW = 128  # id-window capacity per (batch, tile)
NW = BPC * TILES  # 20 windows per core
# matmul N-slices of V2, each within one 2 KiB PSUM bank
NSLICES = [(0, 512), (512, 1024), (1024, 1250)]
NWARM = 16  # tensor-engine warmup matmuls (DVFS clock ramp)
LA = 4  # one-hot build lookahead (tiles)

_cache = {}


def _build(mm_dtype="bfloat16", nwarm=NWARM):
    import concourse.bacc as bacc
    import concourse.mybir as mybir
    import concourse.tile as tile

    f32 = mybir.dt.float32
    f16 = mybir.dt.float16
    mmdt = getattr(mybir.dt, mm_dtype)
    Alu = mybir.AluOpType

    nc = bacc.Bacc("TRN2", target_bir_lowering=False, debug=False, num_devices=NCORES)

    # attn columns gathered per window: [b, p, t*O+o] = attn[b, o, orig_i(b,t,p)]
    attn_d = nc.dram_tensor("attn", [BPC, 128, TILES * O], f32, kind="ExternalInput").ap()
    # hi/lo of span-relative ids: [p, b*TILES+t] (hi = -1 for empty slots)
    hif_d = nc.dram_tensor("hif", [128, NW], f32, kind="ExternalInput").ap()
    lof_d = nc.dram_tensor("lof", [128, NW], f32, kind="ExternalInput").ap()
    out_d = nc.dram_tensor("out", [BPC, O, SIZE], f32, kind="ExternalOutput").ap()

    with tile.TileContext(nc) as tc:
        with (
            tc.tile_pool(name="const", bufs=1) as constp,
            tc.tile_pool(name="idx", bufs=1) as idxp,
            tc.tile_pool(name="gt", bufs=LA + 2) as gtp,
            tc.tile_pool(name="alo", bufs=LA + 2) as alop,
            tc.tile_pool(name="outs", bufs=8) as outp,
            tc.tile_pool(name="psmm", bufs=2, space="PSUM") as psmm,
        ):
            # compare constants built on-device, off the input critical path:
            # lov[p, l] = l; gidx[p, o*4+g] = g
            lov = constp.tile([128, V2], f16, tag="lov")
            nc.gpsimd.iota(lov[:], [[1, V2]], channel_multiplier=0,
                           allow_small_or_imprecise_dtypes=True)
            gidx = constp.tile([128, O * GPT], f16, tag="gidx")
            nc.gpsimd.iota(gidx[:], [[0, O], [1, GPT]], channel_multiplier=0,
                           allow_small_or_imprecise_dtypes=True)

            if nwarm:
                warm = constp.tile([128, 256], mmdt, tag="warm")
                nc.vector.memset(warm[:], 0)
                wps = psmm.tile([128, 256], f32, tag="wm", bufs=1)
                for _ in range(nwarm):
                    nc.tensor.matmul(out=wps[:, :256], lhsT=warm[:, :128],
                                     rhs=warm[:, :256], start=True, stop=True)

            hi_f = idxp.tile([128, NW], f32, tag="hi_f")
            nc.scalar.dma_start(out=hi_f[:], in_=hif_d[:])
            lo_f = idxp.tile([128, NW], f32, tag="lo_f")
            nc.scalar.dma_start(out=lo_f[:], in_=lof_d[:])
            at = []
            for b in range(BPC):
                t_ = constp.tile([128, TILES * O], f32, tag=f"attn{b}", name=f"at{b}")
                nc.sync.dma_start(out=t_[:], in_=attn_d[b])
                at.append(t_)

            # one-hot builds run LA tiles ahead of the matmuls, interleaved
            # with the vector engine's share of the PSUM->SBUF copies, so
            # the matmul -> copy -> DMA pipeline starts immediately and the
            # vector engine is never a serial prefix
            alos, gts = [], []

            def build(w):
                b, t = divmod(w, TILES)
                alo = alop.tile([128, V2], mmdt, tag="alo", name=f"alo{w}")
                nc.vector.tensor_scalar(out=alo[:], in0=lov[:],
                                        scalar1=lo_f[:, w : w + 1],
                                        scalar2=None, op0=Alu.is_equal)
                gt = gtp.tile([128, O * GPT], mmdt, tag="gt", name=f"gt{w}")
                nc.vector.scalar_tensor_tensor(
                    out=gt[:].rearrange("p (o g) -> p o g", g=GPT),
                    in0=gidx[:].rearrange("p (o g) -> p o g", g=GPT),
                    scalar=hi_f[:, w : w + 1],
                    in1=at[b][:, t * O : (t + 1) * O]
                    .unsqueeze(2)
                    .broadcast_to([128, O, GPT]),
                    op0=Alu.is_equal,
                    op1=Alu.mult,
                )
                alos.append(alo)
                gts.append(gt)

            for w in range(LA):
                build(w)

            for w in range(NW):
                if w + LA < NW:
                    build(w + LA)
                b, t = divmod(w, TILES)
                alo, gt = alos[w], gts[w]
                ps = psmm.tile([128, V2], f32, tag="mm")
                for n0, n1 in NSLICES:
                    nc.tensor.matmul(out=ps[:, n0:n1], lhsT=gt[:],
                                     rhs=alo[:, n0:n1], start=True, stop=True)
                os_ = outp.tile([128, V2], f32, tag="os")
                # [32, 4, 1250] view; iteration order (o, g, l) matches
                # the SBUF tile's (partition=(o,g), l) order, and the
                # outer dim of 32 spreads over all 16 SDMA engines
                outv = out_d[b][:, t * SPAN : (t + 1) * SPAN].rearrange(
                    "o (g l) -> o g l", l=V2
                )
                # column-split copy: scalar and vector each move part of
                # every tile (vector also builds, so it gets the smaller
                # share), and each kicks its own half on its own HWDGE queue
                h = 750
                nc.scalar.copy(out=os_[:, :h], in_=ps[:, :h])
                nc.vector.tensor_copy(out=os_[:, h:], in_=ps[:, h:])
                nc.scalar.dma_start(out=outv[:, :, :h], in_=os_[:, :h])
                nc.sync.dma_start(out=outv[:, :, h:], in_=os_[:, h:])

    nc.compile()
    return nc


def _in_maps(ids, attn):
    hi_w = np.full((B, TILES, KW), -1.0, dtype=np.float32)
    lo_w = np.zeros((B, TILES, KW), dtype=np.float32)
    attn_w = np.zeros((B, TILES, KW, O), dtype=np.float32)
    for b in range(B):
        t_of = ids[b] // SPAN
        for t in range(TILES):
            sel = np.nonzero(t_of == t)[0]
            c = sel.size
            if c > KW:
                raise RuntimeError(
                    f"id window overflow: batch {b} span {t} has {c} > {KW} ids"
                )
            rel = ids[b, sel] - t * SPAN
            hi_w[b, t, :c] = rel // V2
            lo_w[b, t, :c] = rel % V2
            attn_w[b, t, :c, :] = attn[b][:, sel].T
    # [8, 128, NW] with [c, p, b*TILES+t]
    hi_t = hi_w.reshape(NCORES, NW, KW).transpose(0, 2, 1)
    lo_t = lo_w.reshape(NCORES, NW, KW).transpose(0, 2, 1)
    attn_t = attn_w.reshape(NCORES, BPC, TILES, KW, O).transpose(
        0, 1, 3, 2, 4
    ).reshape(NCORES, BPC, KW, TILES * O)
    in_maps = [
        {
            "attn": np.ascontiguousarray(attn_t[c]),
            "hif": np.ascontiguousarray(hi_t[c]),
            "lof": np.ascontiguousarray(lo_t[c]),
        }
        for c in range(NCORES)
    ]
    return in_maps


def kernel(ids, attn):
    from concourse.bass_utils import run_bass_kernel_spmd

    ids = np.ascontiguousarray(ids, dtype=np.int32)
    attn = np.ascontiguousarray(attn, dtype=np.float32)

    if "nc" not in _cache:
        _cache["nc"] = _build()
    nc = _cache["nc"]

    core_ids = list(range(NCORES))
    res = run_bass_kernel_spmd(nc, _in_maps(ids, attn), core_ids)
    out = np.concatenate([res.results[c]["out"] for c in core_ids], axis=0)
    return out
